# revision 14
# baseline (speedup 1.0000x reference)
import sys, os
sys.path.insert(0, "/opt/trn_rl_repo")
import numpy as np
import ml_dtypes

from concourse import bass, bacc, tile, mybir
from concourse.bass_utils import run_bass_kernel_spmd

bf16 = mybir.dt.bfloat16
f32 = mybir.dt.float32
i32 = mybir.dt.int32
i16 = mybir.dt.int16
AF = mybir.ActivationFunctionType
ALU = mybir.AluOpType
AX = mybir.AxisListType

NC = 8
H = 128
EPS = 1e-5


def _wrap_idx(a):
    # gather idx layout: token i at [i%16, i//16], replicated to 128 partitions
    n = len(a)
    n16 = (n + 15) // 16
    w = np.zeros((16, n16), np.int16)
    for p in range(16):
        w[p, : len(a[p::16])] = a[p::16]
    return np.tile(w, (8, 1))


def build(cfg):
    N, E, L = cfg["N"], cfg["E"], cfg["L"]
    NPC, NPAD = cfg["NPC"], cfg["NPAD"]
    ECP = cfg["EC_PAD"]
    wsched = cfg["wsched"]          # len ET, window index per 128-edge tile
    NW = NPAD // 128
    NT = NPAD // 128
    ET = ECP // 128
    ECH = ECP // 512                # edge chunks
    assert ET == len(wsched) and ECP % 512 == 0
    TBL = NC * NPAD
    fl = cfg["flags"]

    nc = bacc.Bacc(None, target_bir_lowering=False, num_devices=NC)

    P = lambda n, s, d: nc.declare_dram_parameter(n, s, d, isOutput=False)
    xT_d = P("xT", [5, NPAD], bf16)
    eaT_d = P("eaT", [3, ECP], bf16)
    src_d = P("srci", [128, ECP // 16], i16)
    segB_d = P("segB", [ET, 2, 128, 128], bf16)   # [t,0]=seg (edge->slot), [t,1]=segT
    icntb_d = P("icntb", [128, NT, 128], f32)
    ident_d = P("ident", [128, 128], bf16)
    ones1_d = P("ones1", [1, 128], f32)
    onesK_d = P("onesK", [128, 1], f32)
    encNW0_d = P("encNW0", [5, 128], bf16)
    encNW_d = P("encNW", [3, 128, 128], bf16)
    encEW0_d = P("encEW0", [3, 128], bf16)
    encEW_d = P("encEW", [3, 128, 128], bf16)
    eW0_d = P("eW0", [L, 3, 128, 128], bf16)
    eWs0_d = P("eWs0", [L, 128, 128], bf16)
    eWs1_d = P("eWs1", [L, 128, 128], bf16)
    nW0_d = P("nW0", [L, 2, 128, 128], bf16)
    nWs0_d = P("nWs0", [L, 128, 128], bf16)
    nWs1_d = P("nWs1", [L, 128, 128], bf16)
    decW_d = P("decW", [3, 128, 128], bf16)
    decWl_d = P("decWl", [128, 3], bf16)
    encNb_d = P("encNb", [128, 4], f32)
    encEb_d = P("encEb", [128, 4], f32)
    eb_d = P("eb", [128, 3 * L], f32)
    nb_d = P("nb", [128, 3 * L], f32)
    decb_d = P("decb", [128, 3], f32)
    if fl["eln"]:
        elnw_d = P("elnw", [L, 128, 128], f32)
        elnb_d = P("elnb", [L, 128, 128], f32)
    if fl["nln"]:
        nlnw_d = P("nlnw", [L, 128, 128], f32)
        nlnb_d = P("nlnb", [L, 128, 128], f32)
    if fl["gln"]:
        gNw_d = P("gNw", [128, 128], f32)
        gNb_d = P("gNb", [128, 128], f32)
        gEw_d = P("gEw", [128, 1], f32)
        gEb_d = P("gEb", [128, 1], f32)
    if fl["decbl"]:
        decbl_d = P("decbl", [3, 1], f32)

    out_d = nc.declare_dram_parameter("out", [3, NPAD], f32, isOutput=True)
    DBG = int(os.environ.get("KDBG", "0"))
    if DBG:
        dbg_e_d = nc.declare_dram_parameter("dbg_e", [128, ET, 128], f32, isOutput=True)
        dbg_h_d = nc.declare_dram_parameter("dbg_h", [128, NT, 128], f32, isOutput=True)
        dbg_hfm_d = nc.declare_dram_parameter("dbg_hfm", [128, NT, 128], f32, isOutput=True)
        dbg_hsf_d = nc.declare_dram_parameter("dbg_hsf", [128, 4, 128], f32, isOutput=True)
        dbg_agf_d = nc.declare_dram_parameter("dbg_agf", [128, NT, 128], f32, isOutput=True)
        dbg_h1_d = nc.declare_dram_parameter("dbg_h1", [128, NT, 128], f32, isOutput=True)
        dbg_e1_d = nc.declare_dram_parameter("dbg_e1", [128, ET, 128], f32, isOutput=True)
    hsh_d = nc.dram_tensor("hsh", [NPAD, 128], bf16)
    htab_d = nc.dram_tensor("htab", [TBL, 128], bf16, addr_space="Shared")
    htabl_d = nc.dram_tensor("htabl", [TBL, 128], bf16)
    sti_d = nc.dram_tensor("sti", [4], f32)
    sto_d = nc.dram_tensor("sto", [4], f32, addr_space="Shared")

    RG = [list(range(NC))]

    with tile.TileContext(nc) as tc:
        with (
            tc.tile_pool(name="const", bufs=1) as cp,
            tc.tile_pool(name="big", bufs=1) as bigp,
            tc.tile_pool(name="work", bufs=3) as wp,
            tc.tile_pool(name="segp", bufs=3) as segp,
            tc.tile_pool(name="gath", bufs=3) as gatp,
            tc.tile_pool(name="stat", bufs=4) as sp,
            tc.tile_pool(name="pM", bufs=2, space="PSUM") as pM,
            tc.tile_pool(name="pS", bufs=2, space="PSUM") as pSp,
            tc.tile_pool(name="pT", bufs=2, space="PSUM") as pTp,
            tc.tile_pool(name="pA", bufs=1, space="PSUM") as pAp,
            tc.tile_pool(name="pW", bufs=1, space="PSUM") as pWp,
        ):
            # ---- persistent SBUF state ----
            e_fm = bigp.tile([128, ET, 128], f32)      # edge features, feature-major f32
            h_own = bigp.tile([128, NT, 128], f32)     # node features, row-major f32
            hb = bigp.tile([128, NT, 128], bf16)       # h row-major bf16
            hfm = bigp.tile([128, NT, 128], bf16)      # h feature-major bf16
            agf = bigp.tile([128, NT, 128], bf16)      # aggregated msgs, feature-major bf16

            def ld(shape, dt, src, tag):
                t = cp.tile(shape, dt, tag=tag)
                nc.sync.dma_start(t[:], src[:])
                return t

            def ldw(src, n, tag, dt=bf16):
                t = cp.tile([128, n, 128], dt, tag=tag)
                nc.sync.dma_start(t[:], src[:].rearrange("n k m -> k n m"))
                return t

            xT = ld([5, NPAD], bf16, xT_d, "xT")
            srci = ld([128, ECP // 16], i16, src_d, "srci")
            icntb = cp.tile([128, NT, 128], f32, tag="icntb")
            nc.sync.dma_start(icntb[:], icntb_d[:])
            ident = ld([128, 128], bf16, ident_d, "ident")
            ones1 = ld([1, 128], f32, ones1_d, "ones1")
            onesK = ld([128, 1], f32, onesK_d, "onesK")
            encNW0 = ld([5, 128], bf16, encNW0_d, "encNW0")
            encNW = ldw(encNW_d, 3, "encNW")
            encEW0 = ld([3, 128], bf16, encEW0_d, "encEW0")
            encEW = ldw(encEW_d, 3, "encEW")
            eW0 = cp.tile([128, L * 3, 128], bf16, tag="eW0")
            nc.sync.dma_start(eW0[:], eW0_d[:].rearrange("l n k m -> k (l n) m"))
            eWs0 = ldw(eWs0_d, L, "eWs0")
            eWs1 = ldw(eWs1_d, L, "eWs1")
            nW0 = cp.tile([128, L * 2, 128], bf16, tag="nW0")
            nc.sync.dma_start(nW0[:], nW0_d[:].rearrange("l n k m -> k (l n) m"))
            nWs0 = ldw(nWs0_d, L, "nWs0")
            nWs1 = ldw(nWs1_d, L, "nWs1")
            decW = ldw(decW_d, 3, "decW")
            decWl = ld([128, 3], bf16, decWl_d, "decWl")
            encNb = ld([128, 4], f32, encNb_d, "encNb")
            encEb = ld([128, 4], f32, encEb_d, "encEb")
            eb = ld([128, 3 * L], f32, eb_d, "eb")
            nb = ld([128, 3 * L], f32, nb_d, "nb")
            decb = ld([128, 3], f32, decb_d, "decb")
            if fl["eln"]:
                elnw = ldw(elnw_d, L, "elnw", f32)
                elnb = ldw(elnb_d, L, "elnb", f32)
            if fl["nln"]:
                nlnw = ldw(nlnw_d, L, "nlnw", f32)
                nlnb = ldw(nlnb_d, L, "nlnb", f32)
            if fl["gln"]:
                gNw = ld([128, 128], f32, gNw_d, "gNw")
                gNb = ld([128, 128], f32, gNb_d, "gNb")
                gEw = ld([128, 1], f32, gEw_d, "gEw")
                gEb = ld([128, 1], f32, gEb_d, "gEb")
            if fl["decbl"]:
                decbl = ld([3, 1], f32, decbl_d, "decbl")

            hsum = sp.tile([128, 1], f32, tag="hsum")
            hsq = sp.tile([128, 1], f32, tag="hsq")
            esum = sp.tile([128, 1], f32, tag="esum")
            esq = sp.tile([128, 1], f32, tag="esq")
            for t_ in (hsum, hsq, esum, esq):
                nc.vector.memset(t_[:], 0.0)
            epsA = sp.tile([128, 1], f32, tag="epsA")
            nc.vector.memset(epsA[:], EPS)

            NCH = (NPAD + 511) // 512

            # ================= NODE ENCODER =================
            for c in range(NCH):
                c0 = c * 512
                w = min(512, NPAD - c0)
                nt4 = w // 128
                ps = pM.tile([128, 512], f32, tag="pM")
                nc.tensor.matmul(ps[:, :w], encNW0[:], xT[:, c0 : c0 + w], start=True, stop=True)
                a1 = wp.tile([128, 512], bf16, tag="a1")
                nc.scalar.activation(a1[:, :w], ps[:, :w], AF.Relu, bias=encNb[:, 0:1])
                ps2 = pM.tile([128, 512], f32, tag="pM")
                nc.tensor.matmul(ps2[:, :w], encNW[:, 0, :], a1[:, :w], start=True, stop=True)
                a2 = wp.tile([128, 512], bf16, tag="a2")
                nc.scalar.activation(a2[:, :w], ps2[:, :w], AF.Relu, bias=encNb[:, 1:2])
                ps3 = pM.tile([128, 512], f32, tag="pM")
                nc.tensor.matmul(ps3[:, :w], encNW[:, 1, :], a2[:, :w], start=True, stop=True)
                a3 = wp.tile([128, 512], bf16, tag="a1")
                nc.scalar.activation(a3[:, :w], ps3[:, :w], AF.Relu, bias=encNb[:, 2:3])
                psr = pSp.tile([128, 4, 128], f32, tag="pS")
                for j in range(nt4):
                    nc.tensor.matmul(psr[:, j, :], a3[:, j * 128 : (j + 1) * 128], encNW[:, 2, :], start=True, stop=True)
                s1c = sp.tile([128, 1], f32, tag="s1c")
                nc.scalar.activation(h_own[:, 4 * c : 4 * c + nt4, :], psr[:, :nt4, :], AF.Copy, accum_out=s1c[:])
                scr = wp.tile([128, 512], f32, tag="scr")
                hvv = h_own[:, 4 * c : 4 * c + nt4, :].rearrange("p t f -> p (t f)")
                nc.vector.tensor_tensor(scr[:, :w], hvv, hvv, ALU.mult)
                s2c = sp.tile([128, 1], f32, tag="s2c")
                nc.vector.tensor_reduce(s2c[:], scr[:, :w], AX.X, ALU.add)
                nc.vector.tensor_tensor(hsum[:], hsum[:], s1c[:], ALU.add)
                nc.vector.tensor_tensor(hsq[:], hsq[:], s2c[:], ALU.add)

            # ================= EDGE ENCODER =================
            for c in range(ECH):
                c0 = c * 512
                eat = wp.tile([3, 512], bf16, tag="eat")
                nc.sync.dma_start(eat[:], eaT_d[:, c0 : c0 + 512])
                ps = pM.tile([128, 512], f32, tag="pM")
                nc.tensor.matmul(ps[:], encEW0[:], eat[:], start=True, stop=True)
                a1 = wp.tile([128, 512], bf16, tag="a1")
                nc.scalar.activation(a1[:], ps[:], AF.Relu, bias=encEb[:, 0:1])
                ps2 = pM.tile([128, 512], f32, tag="pM")
                nc.tensor.matmul(ps2[:], encEW[:, 0, :], a1[:], start=True, stop=True)
                a2 = wp.tile([128, 512], bf16, tag="a2")
                nc.scalar.activation(a2[:], ps2[:], AF.Relu, bias=encEb[:, 1:2])
                ps3 = pM.tile([128, 512], f32, tag="pM")
                nc.tensor.matmul(ps3[:], encEW[:, 1, :], a2[:], start=True, stop=True)
                a3 = wp.tile([128, 512], bf16, tag="a1")
                nc.scalar.activation(a3[:], ps3[:], AF.Relu, bias=encEb[:, 2:3])
                ps4 = pM.tile([128, 512], f32, tag="pM")
                nc.tensor.matmul(ps4[:], encEW[:, 2, :], a3[:], start=True, stop=True)
                s1c = sp.tile([128, 1], f32, tag="s1c")
                nc.scalar.activation(e_fm[:, 4 * c : 4 * c + 4, :], ps4[:].rearrange("p (t f) -> p t f", t=4), AF.Copy, accum_out=s1c[:])
                scr = wp.tile([128, 512], f32, tag="scr")
                evv = e_fm[:, 4 * c : 4 * c + 4, :].rearrange("p t f -> p (t f)")
                nc.vector.tensor_tensor(scr[:], evv, evv, ALU.mult)
                s2c = sp.tile([128, 1], f32, tag="s2c")
                nc.vector.tensor_reduce(s2c[:], scr[:], AX.X, ALU.add)
                nc.vector.tensor_tensor(esum[:], esum[:], s1c[:], ALU.add)
                nc.vector.tensor_tensor(esq[:], esq[:], s2c[:], ALU.add)

            # ============ GLOBAL GRAPH-LN STATS ============
            st4 = sp.tile([128, 4], f32, tag="st4")
            for j, t_ in enumerate((hsum, hsq, esum, esq)):
                nc.vector.tensor_copy(st4[:, j : j + 1], t_[:])
            psst = pSp.tile([128, 4, 128], f32, tag="pS")
            nc.tensor.matmul(psst[:4, 0, :1], st4[:], onesK[:], start=True, stop=True)
            stv = sp.tile([4, 1], f32, tag="stv")
            nc.scalar.activation(stv[:], psst[:4, 0, :1], AF.Copy)
            nc.sync.dma_start(sti_d[:], stv[:, 0:1])
            nc.gpsimd.collective_compute(
                "AllReduce", ALU.add, replica_groups=RG, ins=[sti_d[:]], outs=[sto_d[:]]
            )
            st14 = sp.tile([1, 4], f32, tag="st14")
            nc.sync.dma_start(st14[:], sto_d[:])
            psb = pSp.tile([128, 4, 128], f32, tag="pS")
            nc.tensor.matmul(psb[:, 0, :4], ones1[:], st14[:], start=True, stop=True)
            stb = sp.tile([128, 4], f32, tag="stb")
            nc.scalar.activation(stb[:], psb[:, 0, :4], AF.Copy)

            def graph_ln_factors(sumc, sqc, count):
                mu = sp.tile([128, 1], f32, tag="gmu")
                nc.vector.tensor_scalar(mu[:], sumc, 1.0 / count, None, ALU.mult)
                e2 = sp.tile([128, 1], f32, tag="ge2")
                nc.vector.tensor_scalar(e2[:], sqc, 1.0 / count, None, ALU.mult)
                mu2 = sp.tile([128, 1], f32, tag="gmu2")
                nc.scalar.activation(mu2[:], mu[:], AF.Square)
                var = sp.tile([128, 1], f32, tag="gvar")
                nc.vector.tensor_tensor(var[:], e2[:], mu2[:], ALU.subtract)
                sd = sp.tile([128, 1], f32, tag="gsd")
                nc.scalar.activation(sd[:], var[:], AF.Sqrt)
                nc.vector.tensor_scalar(sd[:], sd[:], EPS, None, ALU.add)
                r = sp.tile([128, 1], f32, tag="gr")
                nc.vector.reciprocal(r[:], sd[:])
                nmr = sp.tile([128, 1], f32, tag="gnmr")
                nc.vector.tensor_scalar(nmr[:], mu[:], r[:], -1.0, ALU.mult, ALU.mult)
                return r, nmr

            rh, nmrh = graph_ln_factors(stb[:, 0:1], stb[:, 1:2], float(N) * H)
            re, nmre = graph_ln_factors(stb[:, 2:3], stb[:, 3:4], float(E) * H)

            # apply graph-LN; build hb / hfm / h-table shard
            for c in range(NCH):
                c0 = c * 512
                w = min(512, NPAD - c0)
                nt4 = w // 128
                hv = h_own[:, 4 * c : 4 * c + nt4, :].rearrange("p t f -> p (t f)")
                nc.vector.tensor_scalar(hv, hv, rh[:], nmrh[:], ALU.mult, ALU.add)
                if fl["gln"]:
                    for j in range(nt4):
                        nc.vector.tensor_tensor(h_own[:, 4 * c + j, :], h_own[:, 4 * c + j, :], gNw[:], ALU.mult)
                        nc.vector.tensor_tensor(h_own[:, 4 * c + j, :], h_own[:, 4 * c + j, :], gNb[:], ALU.add)
                nc.scalar.activation(hb[:, 4 * c : 4 * c + nt4, :], h_own[:, 4 * c : 4 * c + nt4, :], AF.Copy)
                nc.sync.dma_start(
                    hsh_d[c0 : c0 + w, :].rearrange("(t p) f -> p t f", p=128),
                    hb[:, 4 * c : 4 * c + nt4, :],
                )
                nc.scalar.dma_start_transpose(
                    hfm[:, 4 * c : 4 * c + nt4, :],
                    hb[:, 4 * c : 4 * c + nt4, :].rearrange("p t f -> p (t f)"),
                )
            for c in range(ECH):
                evc = e_fm[:, 4 * c : 4 * c + 4, :].rearrange("p t f -> p (t f)")
                nc.vector.tensor_scalar(evc, evc, re[:], nmre[:], ALU.mult, ALU.add)
                if fl["gln"]:
                    nc.vector.tensor_scalar(evc, evc, gEw[:], gEb[:], ALU.mult, ALU.add)
            nc.gpsimd.collective_compute(
                "AllGather", ALU.bypass, replica_groups=RG, ins=[hsh_d[:]], outs=[htab_d[:]]
            )
            nc.sync.dma_start(htabl_d[:], htab_d[:])
            if DBG:
                nc.sync.dma_start(dbg_e_d[:], e_fm[:])
                nc.sync.dma_start(dbg_h_d[:], h_own[:])
                dcp = wp.tile([128, NT, 128], f32, tag="dcp")
                nc.scalar.activation(dcp[:], hfm[:], AF.Copy)
                nc.sync.dma_start(dbg_hfm_d[:], dcp[:])

            # ================= MP LAYERS =================
            for l in range(L):
                cur_w = -1
                pagg = None
                pw_sbs = {}
                ntile_w = {}
                for t in range(ET):
                    ntile_w[wsched[t]] = ntile_w.get(wsched[t], 0) + 1
                seen_w = {}

                # -------- edge phase --------
                for c in range(ECH):
                    hsr = gatp.tile([128, 4, 128], bf16, tag="hsr")
                    nc.gpsimd.dma_gather(hsr[:], htabl_d[:], srci[:, c * 32 : c * 32 + 32], 512, 512, 128, transpose=False)
                    pth = pTp.tile([128, 4, 128], bf16, tag="pT")
                    for j in range(4):
                        nc.tensor.transpose(pth[:, j, :], hsr[:, j, :], ident[:])
                    hsf = gatp.tile([128, 4, 128], bf16, tag="hsf")
                    nc.scalar.activation(hsf[:], pth[:], AF.Copy)
                    if DBG and l == 0 and c == 0:
                        dcph = wp.tile([128, 4, 128], f32, tag="dcph")
                        nc.scalar.activation(dcph[:], hsf[:], AF.Copy)
                        nc.sync.dma_start(dbg_hsf_d[:], dcph[:])
                    segB = segp.tile([128, 8, 128], bf16, tag="segB")
                    nc.sync.dma_start(segB[:], segB_d[4 * c : 4 * c + 4].rearrange("t s p f -> p (t s) f"))
                    ebf = wp.tile([128, 512], bf16, tag="ebf")
                    nc.vector.tensor_copy(ebf[:], e_fm[:, 4 * c : 4 * c + 4, :].rearrange("p t f -> p (t f)"))
                    ps = pM.tile([128, 512], f32, tag="pM")
                    nc.tensor.matmul(ps[:], eW0[:, 3 * l + 1, :], hsf[:].rearrange("p t f -> p (t f)"), start=True, stop=False, skip_group_check=True)
                    nc.tensor.matmul(ps[:], eW0[:, 3 * l + 2, :], ebf[:], start=False, stop=False, skip_group_check=True)
                    for j in range(4):
                        t = 4 * c + j
                        wd = wsched[t]
                        if wd not in pw_sbs:
                            psw = pWp.tile([128, 128], f32, tag="pW")
                            nc.tensor.matmul(psw[:], hfm[:, wd, :], eW0[:, 3 * l, :], start=True, stop=True)
                            pw_sbs[wd] = wp.tile([128, 128], bf16, tag="pwsb", name="pwsb")
                            nc.scalar.activation(pw_sbs[wd][:], psw[:], AF.Copy)
                        nc.tensor.matmul(ps[:, j * 128 : (j + 1) * 128], pw_sbs[wd][:], segB[:, 2 * j + 1, :], start=False, stop=True, skip_group_check=True)
                    a1 = wp.tile([128, 512], bf16, tag="a1")
                    nc.scalar.activation(a1[:], ps[:], AF.Relu, bias=eb[:, 3 * l : 3 * l + 1])
                    ps2 = pM.tile([128, 512], f32, tag="pM")
                    nc.tensor.matmul(ps2[:], eWs0[:, l, :], a1[:], start=True, stop=True)
                    a2 = wp.tile([128, 512], bf16, tag="a2")
                    nc.scalar.activation(a2[:], ps2[:], AF.Relu, bias=eb[:, 3 * l + 1 : 3 * l + 2])
                    ps3 = pSp.tile([128, 4, 128], f32, tag="pS")
                    for j in range(4):
                        nc.tensor.matmul(ps3[:, j, :], a2[:, j * 128 : (j + 1) * 128], eWs1[:, l, :], start=True, stop=True)
                    # batched row-LN stats over the 4 tiles
                    p3f = ps3[:].rearrange("p t f -> p (t f)")
                    s1 = sp.tile([128, 4], f32, tag="s1")
                    nc.vector.tensor_reduce(s1[:], ps3[:], AX.X, ALU.add)
                    scr = wp.tile([128, 512], f32, tag="scr")
                    nc.scalar.activation(scr[:], p3f, AF.Square)
                    s2 = sp.tile([128, 4], f32, tag="s2")
                    nc.vector.tensor_reduce(s2[:], scr[:].rearrange("p (t f) -> p t f", t=4), AX.X, ALU.add)
                    mu = sp.tile([128, 4], f32, tag="mu")
                    nc.vector.tensor_scalar(mu[:], s1[:], 1.0 / 128, None, ALU.mult)
                    es2 = sp.tile([128, 4], f32, tag="es2")
                    nc.vector.tensor_scalar(es2[:], s2[:], 1.0 / 128, None, ALU.mult)
                    mu2 = sp.tile([128, 4], f32, tag="mu2")
                    nc.vector.tensor_tensor(mu2[:], mu[:], mu[:], ALU.mult)
                    var = sp.tile([128, 4], f32, tag="var")
                    nc.vector.tensor_tensor(var[:], es2[:], mu2[:], ALU.subtract)
                    sd = sp.tile([128, 4], f32, tag="sd")
                    nc.scalar.activation(sd[:], var[:], AF.Sqrt, bias=epsA[:])
                    rs = sp.tile([128, 4], f32, tag="rs")
                    nc.vector.reciprocal(rs[:], sd[:])
                    nmr = sp.tile([128, 4], f32, tag="nmr")
                    nc.vector.tensor_tensor(nmr[:], mu[:], rs[:], ALU.mult)
                    nc.vector.tensor_scalar(nmr[:], nmr[:], -1.0, None, ALU.mult)
                    tmpb4 = wp.tile([128, 4, 128], bf16, tag="tmpb4")
                    for j in range(4):
                        t = 4 * c + j
                        wd = wsched[t]
                        nc.vector.tensor_scalar(tmpb4[:, j, :], ps3[:, j, :], rs[:, j : j + 1], nmr[:, j : j + 1], ALU.mult, ALU.add)
                        if fl["eln"]:
                            tmpf = wp.tile([128, 128], f32, tag="tmpf")
                            nc.vector.tensor_scalar(tmpf[:], ps3[:, j, :], rs[:, j : j + 1], nmr[:, j : j + 1], ALU.mult, ALU.add)
                            nc.vector.tensor_tensor(tmpf[:], tmpf[:], elnw[:, l, :], ALU.mult)
                            nc.vector.tensor_tensor(tmpf[:], tmpf[:], elnb[:, l, :], ALU.add)
                            nc.vector.tensor_copy(tmpb4[:, j, :], tmpf[:])
                        if wd != cur_w:
                            if cur_w >= 0:
                                nc.vector.tensor_tensor(agf[:, cur_w, :], pagg[:], icntb[:, cur_w, :], ALU.mult)
                            cur_w = wd
                            pagg = pAp.tile([128, 128], f32, tag="pA")
                            seen_w[wd] = 0
                        nc.tensor.matmul(pagg[:], tmpb4[:, j, :], segB[:, 2 * j, :], start=(seen_w[wd] == 0), stop=(seen_w[wd] == ntile_w[wd] - 1), skip_group_check=True)
                        seen_w[wd] += 1
                    ptr = pTp.tile([128, 4, 128], bf16, tag="pT")
                    for j in range(4):
                        nc.tensor.transpose(ptr[:, j, :], tmpb4[:, j, :], ident[:])
                    ev4 = e_fm[:, 4 * c : 4 * c + 4, :].rearrange("p t f -> p (t f)")
                    nc.vector.tensor_tensor(ev4, ev4, ptr[:].rearrange("p t f -> p (t f)"), ALU.add)
                # close last window
                nc.vector.tensor_tensor(agf[:, cur_w, :], pagg[:], icntb[:, cur_w, :], ALU.mult)
                if DBG and l == 0:
                    dcp2 = wp.tile([128, NT, 128], f32, tag="dcp")
                    nc.scalar.activation(dcp2[:], agf[:], AF.Copy)
                    nc.sync.dma_start(dbg_agf_d[:], dcp2[:])
                    nc.sync.dma_start(dbg_e1_d[:], e_fm[:])

                # -------- node phase --------
                for c in range(NCH):
                    c0 = c * 512
                    w = min(512, NPAD - c0)
                    nt4 = w // 128
                    ps = pM.tile([128, 512], f32, tag="pM")
                    nc.tensor.matmul(ps[:, :w], nW0[:, 2 * l, :], hfm[:, 4 * c : 4 * c + nt4, :].rearrange("p t f -> p (t f)"), start=True, stop=False, skip_group_check=True)
                    nc.tensor.matmul(ps[:, :w], nW0[:, 2 * l + 1, :], agf[:, 4 * c : 4 * c + nt4, :].rearrange("p t f -> p (t f)"), start=False, stop=True, skip_group_check=True)
                    a1 = wp.tile([128, 512], bf16, tag="a1")
                    nc.scalar.activation(a1[:, :w], ps[:, :w], AF.Relu, bias=nb[:, 3 * l : 3 * l + 1])
                    ps2 = pM.tile([128, 512], f32, tag="pM")
                    nc.tensor.matmul(ps2[:, :w], nWs0[:, l, :], a1[:, :w], start=True, stop=True)
                    a2 = wp.tile([128, 512], bf16, tag="a2")
                    nc.scalar.activation(a2[:, :w], ps2[:, :w], AF.Relu, bias=nb[:, 3 * l + 1 : 3 * l + 2])
                    ps3 = pSp.tile([128, 4, 128], f32, tag="pS")
                    for j in range(nt4):
                        nc.tensor.matmul(ps3[:, j, :], a2[:, j * 128 : (j + 1) * 128], nWs1[:, l, :], start=True, stop=True)
                    s1 = sp.tile([128, 4], f32, tag="s1")
                    nc.vector.tensor_reduce(s1[:, :nt4], ps3[:, :nt4, :], AX.X, ALU.add)
                    scr = wp.tile([128, 512], f32, tag="scr")
                    p3f = ps3[:, :nt4, :].rearrange("p t f -> p (t f)")
                    nc.scalar.activation(scr[:, :w], p3f, AF.Square)
                    s2 = sp.tile([128, 4], f32, tag="s2")
                    nc.vector.tensor_reduce(s2[:, :nt4], scr[:, :w].rearrange("p (t f) -> p t f", t=nt4), AX.X, ALU.add)
                    mu = sp.tile([128, 4], f32, tag="mu")
                    nc.vector.tensor_scalar(mu[:, :nt4], s1[:, :nt4], 1.0 / 128, None, ALU.mult)
                    es2 = sp.tile([128, 4], f32, tag="es2")
                    nc.vector.tensor_scalar(es2[:, :nt4], s2[:, :nt4], 1.0 / 128, None, ALU.mult)
                    mu2 = sp.tile([128, 4], f32, tag="mu2")
                    nc.vector.tensor_tensor(mu2[:, :nt4], mu[:, :nt4], mu[:, :nt4], ALU.mult)
                    var = sp.tile([128, 4], f32, tag="var")
                    nc.vector.tensor_tensor(var[:, :nt4], es2[:, :nt4], mu2[:, :nt4], ALU.subtract)
                    sd = sp.tile([128, 4], f32, tag="sd")
                    nc.scalar.activation(sd[:, :nt4], var[:, :nt4], AF.Sqrt, bias=epsA[:])
                    rs = sp.tile([128, 4], f32, tag="rs")
                    nc.vector.reciprocal(rs[:, :nt4], sd[:, :nt4])
                    nmr = sp.tile([128, 4], f32, tag="nmr")
                    nc.vector.tensor_tensor(nmr[:, :nt4], mu[:, :nt4], rs[:, :nt4], ALU.mult)
                    nc.vector.tensor_scalar(nmr[:, :nt4], nmr[:, :nt4], -1.0, None, ALU.mult)
                    upd = wp.tile([128, 4, 128], f32, tag="upd")
                    for j in range(nt4):
                        nc.vector.tensor_scalar(upd[:, j, :], ps3[:, j, :], rs[:, j : j + 1], nmr[:, j : j + 1], ALU.mult, ALU.add)
                        if fl["nln"]:
                            nc.vector.tensor_tensor(upd[:, j, :], upd[:, j, :], nlnw[:, l, :], ALU.mult)
                            nc.vector.tensor_tensor(upd[:, j, :], upd[:, j, :], nlnb[:, l, :], ALU.add)
                    hv = h_own[:, 4 * c : 4 * c + nt4, :].rearrange("p t f -> p (t f)")
                    nc.gpsimd.tensor_tensor(hv, hv, upd[:, :nt4, :].rearrange("p t f -> p (t f)"), ALU.add)
                    nc.scalar.activation(hb[:, 4 * c : 4 * c + nt4, :], h_own[:, 4 * c : 4 * c + nt4, :], AF.Copy)
                    if l < L - 1:
                        nc.sync.dma_start(
                            hsh_d[c0 : c0 + w, :].rearrange("(t p) f -> p t f", p=128),
                            hb[:, 4 * c : 4 * c + nt4, :],
                        )
                    nc.scalar.dma_start_transpose(
                        hfm[:, 4 * c : 4 * c + nt4, :],
                        hb[:, 4 * c : 4 * c + nt4, :].rearrange("p t f -> p (t f)"),
                    )
                if DBG and l == 0:
                    nc.sync.dma_start(dbg_h1_d[:], h_own[:])
                if l < L - 1:
                    nc.gpsimd.collective_compute(
                        "AllGather", ALU.bypass, replica_groups=RG, ins=[hsh_d[:]], outs=[htab_d[:]]
                    )
                    nc.sync.dma_start(htabl_d[:], htab_d[:])

            # ================= DECODER =================
            for c in range(NCH):
                c0 = c * 512
                w = min(512, NPAD - c0)
                nt4 = w // 128
                hfc = hfm[:, 4 * c : 4 * c + nt4, :].rearrange("p t f -> p (t f)")
                ps = pM.tile([128, 512], f32, tag="pM")
                nc.tensor.matmul(ps[:, :w], decW[:, 0, :], hfc, start=True, stop=True)
                a1 = wp.tile([128, 512], bf16, tag="a1")
                nc.scalar.activation(a1[:, :w], ps[:, :w], AF.Relu, bias=decb[:, 0:1])
                ps2 = pM.tile([128, 512], f32, tag="pM")
                nc.tensor.matmul(ps2[:, :w], decW[:, 1, :], a1[:, :w], start=True, stop=True)
                a2 = wp.tile([128, 512], bf16, tag="a2")
                nc.scalar.activation(a2[:, :w], ps2[:, :w], AF.Relu, bias=decb[:, 1:2])
                ps3 = pM.tile([128, 512], f32, tag="pM")
                nc.tensor.matmul(ps3[:, :w], decW[:, 2, :], a2[:, :w], start=True, stop=True)
                a3 = wp.tile([128, 512], bf16, tag="a1")
                nc.scalar.activation(a3[:, :w], ps3[:, :w], AF.Relu, bias=decb[:, 2:3])
                psd = pM.tile([128, 512], f32, tag="pM")
                nc.tensor.matmul(psd[:3, :w], decWl[:], a3[:, :w], start=True, stop=True)
                ot = wp.tile([3, 512], f32, tag="ot")
                if fl["decbl"]:
                    nc.vector.tensor_scalar(ot[:, :w], psd[:3, :w], 1.0, decbl[:], ALU.mult, ALU.add)
                else:
                    nc.scalar.activation(ot[:, :w], psd[:3, :w], AF.Copy)
                nc.sync.dma_start(out_d[:, c0 : c0 + w], ot[:, :w])

    nc.compile()
    return nc


def _prep(inputs, cfg):
    """Host-side sharding/index prep. Returns in_maps list."""
    N, E, L = cfg["N"], cfg["E"], cfg["L"]
    NPC, NPAD, ECP = cfg["NPC"], cfg["NPAD"], cfg["EC_PAD"]
    wsched = cfg["wsched"]
    ET = ECP // 128
    NW = NPAD // 128
    f = lambda k: np.asarray(inputs[k], np.float32)
    b = lambda a: np.ascontiguousarray(a).astype(ml_dtypes.bfloat16)

    ei = np.asarray(inputs["edge_index"])
    src_g, dst_g = ei[0].astype(np.int64), ei[1].astype(np.int64)
    ea = f("edge_attr")
    x = f("x")
    cnt = np.bincount(dst_g, minlength=N).astype(np.float32)
    icnt_full = 1.0 / np.maximum(cnt, 1.0)

    tblrow = lambda g: (g // NPC) * NPAD + (g % NPC)

    pos = {}
    for t, wd in enumerate(wsched):
        pos.setdefault(wd, []).append(t)

    order = np.argsort(dst_g, kind="stable")
    in_maps = []
    shared = None
    for c in range(NC):
        lo, hi = c * NPC, (c + 1) * NPC
        sel = order[(dst_g[order] >= lo) & (dst_g[order] < hi)]
        dl = dst_g[sel] - lo           # local dst, ascending
        win = dl // 128
        srcv = np.zeros(ECP, np.int64)
        eav = np.zeros((ECP, 3), np.float32)
        seg = np.zeros((ET, 128, 128), np.float32)
        for wd in range(NW):
            idxs = np.where(win == wd)[0]
            tiles = pos.get(wd, [])
            assert len(idxs) <= len(tiles) * 128, (c, wd, len(idxs), len(tiles))
            for k, i in enumerate(idxs):
                t = tiles[k // 128]
                r = k % 128
                g = t * 128 + r
                e_ = sel[i]
                srcv[g] = src_g[e_]
                eav[g] = ea[e_]
                seg[t, r, dl[i] - 128 * wd] = 1.0
        segB = np.stack([seg, seg.transpose(0, 2, 1)], axis=1)  # [ET, 2, 128, 128]
        icnt_c = np.ones(NPAD, np.float32)
        icnt_c[:NPC] = icnt_full[lo:hi]
        icntb = np.tile(icnt_c.reshape(1, NW, 128), (128, 1, 1)).copy()
        xT = np.zeros((5, NPAD), np.float32)
        xT[:, :NPC] = x[lo:hi].T
        eaT = eav.T.copy()
        m = {
            "xT": b(xT), "eaT": b(eaT),
            "srci": _wrap_idx(tblrow(srcv).astype(np.int16)),
            "segB": b(segB), "icntb": icntb,
        }
        if shared is None:
            shared = {
                "ident": b(np.eye(128)),
                "ones1": np.ones((1, 128), np.float32),
                "onesK": np.ones((128, 1), np.float32),
                "encNW0": b(f("encN_W0")), "encNW": b(f("encN_Ws")),
                "encEW0": b(f("encE_W0")), "encEW": b(f("encE_Ws")),
                "eW0": b(f("eW0").reshape(L, 3, 128, 128)),
                "eWs0": b(f("eWs")[:, 0]), "eWs1": b(f("eWs")[:, 1]),
                "nW0": b(f("nW0").reshape(L, 2, 128, 128)),
                "nWs0": b(f("nWs")[:, 0]), "nWs1": b(f("nWs")[:, 1]),
                "decW": b(np.stack([f("dec_W0"), f("dec_Ws")[0], f("dec_Ws")[1]])),
                "decWl": b(f("dec_Wl")),
                "encNb": f("encN_bs").T.copy(), "encEb": f("encE_bs").T.copy(),
                "eb": f("ebs").reshape(L * 3, 128).T.copy(),
                "nb": f("nbs").reshape(L * 3, 128).T.copy(),
                "decb": f("dec_bs").T.copy(),
            }
            flg = cfg["flags"]
            if flg["eln"]:
                shared["elnw"] = np.tile(f("elnw")[:, None, :], (1, 128, 1))
                shared["elnb"] = np.tile(f("elnb")[:, None, :], (1, 128, 1))
            if flg["nln"]:
                shared["nlnw"] = np.tile(f("nlnw")[:, None, :], (1, 128, 1))
                shared["nlnb"] = np.tile(f("nlnb")[:, None, :], (1, 128, 1))
            if flg["gln"]:
                shared["gNw"] = np.tile(f("encN_lnw")[None, :], (128, 1))
                shared["gNb"] = np.tile(f("encN_lnb")[None, :], (128, 1))
                shared["gEw"] = f("encE_lnw").reshape(128, 1).copy()
                shared["gEb"] = f("encE_lnb").reshape(128, 1).copy()
            if flg["decbl"]:
                shared["decbl"] = f("dec_bl").reshape(3, 1).copy()
        m.update(shared)
        in_maps.append(m)
    return in_maps


def make_cfg(inputs):
    N = np.asarray(inputs["x"]).shape[0]
    E = np.asarray(inputs["edge_index"]).shape[1]
    L = np.asarray(inputs["eW0"]).shape[0]
    NPC = N // NC
    NPAD = ((NPC + 127) // 128) * 128
    NW = NPAD // 128
    ei = np.asarray(inputs["edge_index"])
    dst = ei[1].astype(np.int64)
    tw = []
    for wd in range(NW):
        mx = 1
        for c in range(NC):
            lo = c * NPC
            nwin = int(((dst >= lo + wd * 128) & (dst < min(lo + (wd + 1) * 128, lo + NPC))).sum())
            mx = max(mx, (nwin + 127) // 128)
        tw.append(mx)
    wsched = []
    for wd in range(NW):
        wsched += [wd] * tw[wd]
    while (len(wsched) * 128) % 512:
        wsched.append(NW - 1)
    flags = {
        "eln": bool(np.any(np.asarray(inputs["elnw"]) != 1) or np.any(np.asarray(inputs["elnb"]) != 0)),
        "nln": bool(np.any(np.asarray(inputs["nlnw"]) != 1) or np.any(np.asarray(inputs["nlnb"]) != 0)),
        "gln": bool(
            np.any(np.asarray(inputs["encN_lnw"]) != 1) or np.any(np.asarray(inputs["encN_lnb"]) != 0)
            or np.any(np.asarray(inputs["encE_lnw"]) != 1) or np.any(np.asarray(inputs["encE_lnb"]) != 0)
        ),
        "decbl": bool(np.any(np.asarray(inputs["dec_bl"]) != 0)),
    }
    return {
        "N": N, "E": E, "L": L, "NPC": NPC, "NPAD": NPAD,
        "EC_PAD": len(wsched) * 128, "wsched": wsched, "flags": flags,
    }


_CACHE = {}


def kernel(**inputs) -> np.ndarray:
    cfg = make_cfg(inputs)
    key = (cfg["N"], cfg["E"], cfg["L"], cfg["EC_PAD"], tuple(sorted(cfg["flags"].items())), os.environ.get("KDBG", "0"))
    if key not in _CACHE:
        _CACHE[key] = build(cfg)
    nc = _CACHE[key]
    in_maps = _prep(inputs, cfg)
    res = run_bass_kernel_spmd(nc, in_maps, list(range(NC))).results
    NPC = cfg["NPC"]
    out = np.concatenate([res[c]["out"][:, :NPC].T for c in range(NC)], axis=0)
    return np.ascontiguousarray(out).astype(np.float32)


# revision 15
# speedup vs baseline: 1.5898x; 1.5898x over previous
import sys, os
sys.path.insert(0, "/opt/trn_rl_repo")
import numpy as np
import ml_dtypes

from concourse import bass, bacc, tile, mybir
from concourse.bass_utils import run_bass_kernel_spmd

bf16 = mybir.dt.bfloat16
f32 = mybir.dt.float32
i32 = mybir.dt.int32
i16 = mybir.dt.int16
AF = mybir.ActivationFunctionType
ALU = mybir.AluOpType
AX = mybir.AxisListType

NC = 8
H = 128
EPS = 1e-5


def _wrap_idx(a):
    # gather idx layout: token i at [i%16, i//16], replicated to 128 partitions
    n = len(a)
    n16 = (n + 15) // 16
    w = np.zeros((16, n16), np.int16)
    for p in range(16):
        w[p, : len(a[p::16])] = a[p::16]
    return np.tile(w, (8, 1))


def build(cfg):
    N, E, L = cfg["N"], cfg["E"], cfg["L"]
    NPC, NPAD = cfg["NPC"], cfg["NPAD"]
    ECP = cfg["EC_PAD"]
    wsched = cfg["wsched"]          # len ET, window index per 128-edge tile
    NW = NPAD // 128
    NT = NPAD // 128
    ET = ECP // 128
    ECH = ECP // 512                # edge chunks
    assert ET == len(wsched) and ECP % 512 == 0
    TBL = NC * NPAD
    fl = cfg["flags"]

    nc = bacc.Bacc(None, target_bir_lowering=False, num_devices=NC, num_swdge_queues=4)

    P = lambda n, s, d: nc.declare_dram_parameter(n, s, d, isOutput=False)
    xT_d = P("xT", [5, NPAD], bf16)
    eaT_d = P("eaT", [3, ECP], bf16)
    src_d = P("srci", [128, ECP // 16], i16)
    segB_d = P("segB", [ET, 2, 128, 128], bf16)   # [t,0]=seg (edge->slot), [t,1]=segT
    icntb_d = P("icntb", [128, NT, 128], f32)
    ident_d = P("ident", [128, 128], bf16)
    ones1_d = P("ones1", [1, 128], f32)
    onesK_d = P("onesK", [128, 1], f32)
    encNW0_d = P("encNW0", [5, 128], bf16)
    encNW_d = P("encNW", [3, 128, 128], bf16)
    encEW0_d = P("encEW0", [3, 128], bf16)
    encEW_d = P("encEW", [3, 128, 128], bf16)
    eW0_d = P("eW0", [L, 3, 128, 128], bf16)
    eWs0_d = P("eWs0", [L, 128, 128], bf16)
    eWs1_d = P("eWs1", [L, 128, 128], bf16)
    nW0_d = P("nW0", [L, 2, 128, 128], bf16)
    nWs0_d = P("nWs0", [L, 128, 128], bf16)
    nWs1_d = P("nWs1", [L, 128, 128], bf16)
    decW_d = P("decW", [3, 128, 128], bf16)
    decWl_d = P("decWl", [128, 3], bf16)
    encNb_d = P("encNb", [128, 4], f32)
    encEb_d = P("encEb", [128, 4], f32)
    eb_d = P("eb", [128, 3 * L], f32)
    nb_d = P("nb", [128, 3 * L], f32)
    decb_d = P("decb", [128, 3], f32)
    if fl["eln"]:
        elnw_d = P("elnw", [L, 128, 128], f32)
        elnb_d = P("elnb", [L, 128, 128], f32)
    if fl["nln"]:
        nlnw_d = P("nlnw", [L, 128, 128], f32)
        nlnb_d = P("nlnb", [L, 128, 128], f32)
    if fl["gln"]:
        gNw_d = P("gNw", [128, 128], f32)
        gNb_d = P("gNb", [128, 128], f32)
        gEw_d = P("gEw", [128, 1], f32)
        gEb_d = P("gEb", [128, 1], f32)
    if fl["decbl"]:
        decbl_d = P("decbl", [3, 1], f32)

    out_d = nc.declare_dram_parameter("out", [3, NPAD], f32, isOutput=True)
    DBG = int(os.environ.get("KDBG", "0"))
    if DBG:
        dbg_e_d = nc.declare_dram_parameter("dbg_e", [128, ET, 128], f32, isOutput=True)
        dbg_h_d = nc.declare_dram_parameter("dbg_h", [128, NT, 128], f32, isOutput=True)
        dbg_hfm_d = nc.declare_dram_parameter("dbg_hfm", [128, NT, 128], f32, isOutput=True)
        dbg_hsf_d = nc.declare_dram_parameter("dbg_hsf", [128, 4, 128], f32, isOutput=True)
        dbg_agf_d = nc.declare_dram_parameter("dbg_agf", [128, NT, 128], f32, isOutput=True)
        dbg_h1_d = nc.declare_dram_parameter("dbg_h1", [128, NT, 128], f32, isOutput=True)
        dbg_e1_d = nc.declare_dram_parameter("dbg_e1", [128, ET, 128], f32, isOutput=True)
    hsh_d = nc.dram_tensor("hsh", [NPAD, 128], bf16)
    htab_d = nc.dram_tensor("htab", [TBL, 128], bf16, addr_space="Shared")
    htabl_d = nc.dram_tensor("htabl", [TBL, 128], bf16)
    sti_d = nc.dram_tensor("sti", [4], f32)
    sto_d = nc.dram_tensor("sto", [4], f32, addr_space="Shared")

    RG = [list(range(NC))]

    with tile.TileContext(nc) as tc:
        with (
            tc.tile_pool(name="const", bufs=1) as cp,
            tc.tile_pool(name="big", bufs=1) as bigp,
            tc.tile_pool(name="work", bufs=3) as wp,
            tc.tile_pool(name="segp", bufs=3) as segp,
            tc.tile_pool(name="gath", bufs=3) as gatp,
            tc.tile_pool(name="stat", bufs=4) as sp,
            tc.tile_pool(name="pM", bufs=2, space="PSUM") as pM,
            tc.tile_pool(name="pS", bufs=2, space="PSUM") as pSp,
            tc.tile_pool(name="pT", bufs=2, space="PSUM") as pTp,
            tc.tile_pool(name="pA", bufs=1, space="PSUM") as pAp,
            tc.tile_pool(name="pW", bufs=1, space="PSUM") as pWp,
        ):
            # ---- persistent SBUF state ----
            e_fm = bigp.tile([128, ET, 128], f32)      # edge features, feature-major f32
            h_own = bigp.tile([128, NT, 128], f32)     # node features, row-major f32
            hb = bigp.tile([128, NT, 128], bf16)       # h row-major bf16
            hfm = bigp.tile([128, NT, 128], bf16)      # h feature-major bf16
            agf = bigp.tile([128, NT, 128], bf16)      # aggregated msgs, feature-major bf16

            def ld(shape, dt, src, tag):
                t = cp.tile(shape, dt, tag=tag)
                nc.sync.dma_start(t[:], src[:])
                return t

            def ldw(src, n, tag, dt=bf16):
                t = cp.tile([128, n, 128], dt, tag=tag)
                nc.sync.dma_start(t[:], src[:].rearrange("n k m -> k n m"))
                return t

            xT = ld([5, NPAD], bf16, xT_d, "xT")
            srci = ld([128, ECP // 16], i16, src_d, "srci")
            icntb = cp.tile([128, NT, 128], f32, tag="icntb")
            nc.sync.dma_start(icntb[:], icntb_d[:])
            ident = ld([128, 128], bf16, ident_d, "ident")
            ones1 = ld([1, 128], f32, ones1_d, "ones1")
            onesK = ld([128, 1], f32, onesK_d, "onesK")
            encNW0 = ld([5, 128], bf16, encNW0_d, "encNW0")
            encNW = ldw(encNW_d, 3, "encNW")
            encEW0 = ld([3, 128], bf16, encEW0_d, "encEW0")
            encEW = ldw(encEW_d, 3, "encEW")
            eW0 = cp.tile([128, L * 3, 128], bf16, tag="eW0")
            nc.sync.dma_start(eW0[:], eW0_d[:].rearrange("l n k m -> k (l n) m"))
            eWs0 = ldw(eWs0_d, L, "eWs0")
            eWs1 = ldw(eWs1_d, L, "eWs1")
            nW0 = cp.tile([128, L * 2, 128], bf16, tag="nW0")
            nc.sync.dma_start(nW0[:], nW0_d[:].rearrange("l n k m -> k (l n) m"))
            nWs0 = ldw(nWs0_d, L, "nWs0")
            nWs1 = ldw(nWs1_d, L, "nWs1")
            decW = ldw(decW_d, 3, "decW")
            decWl = ld([128, 3], bf16, decWl_d, "decWl")
            encNb = ld([128, 4], f32, encNb_d, "encNb")
            encEb = ld([128, 4], f32, encEb_d, "encEb")
            eb = ld([128, 3 * L], f32, eb_d, "eb")
            nb = ld([128, 3 * L], f32, nb_d, "nb")
            decb = ld([128, 3], f32, decb_d, "decb")
            if fl["eln"]:
                elnw = ldw(elnw_d, L, "elnw", f32)
                elnb = ldw(elnb_d, L, "elnb", f32)
            if fl["nln"]:
                nlnw = ldw(nlnw_d, L, "nlnw", f32)
                nlnb = ldw(nlnb_d, L, "nlnb", f32)
            if fl["gln"]:
                gNw = ld([128, 128], f32, gNw_d, "gNw")
                gNb = ld([128, 128], f32, gNb_d, "gNb")
                gEw = ld([128, 1], f32, gEw_d, "gEw")
                gEb = ld([128, 1], f32, gEb_d, "gEb")
            if fl["decbl"]:
                decbl = ld([3, 1], f32, decbl_d, "decbl")

            hsum = sp.tile([128, 1], f32, tag="hsum")
            hsq = sp.tile([128, 1], f32, tag="hsq")
            esum = sp.tile([128, 1], f32, tag="esum")
            esq = sp.tile([128, 1], f32, tag="esq")
            for t_ in (hsum, hsq, esum, esq):
                nc.vector.memset(t_[:], 0.0)
            epsA = sp.tile([128, 1], f32, tag="epsA")
            nc.vector.memset(epsA[:], EPS)

            NCH = (NPAD + 511) // 512

            # ================= NODE ENCODER =================
            for c in range(NCH):
                c0 = c * 512
                w = min(512, NPAD - c0)
                nt4 = w // 128
                ps = pM.tile([128, 512], f32, tag="pM")
                nc.tensor.matmul(ps[:, :w], encNW0[:], xT[:, c0 : c0 + w], start=True, stop=True)
                a1 = wp.tile([128, 512], bf16, tag="a1")
                nc.scalar.activation(a1[:, :w], ps[:, :w], AF.Relu, bias=encNb[:, 0:1])
                ps2 = pM.tile([128, 512], f32, tag="pM")
                nc.tensor.matmul(ps2[:, :w], encNW[:, 0, :], a1[:, :w], start=True, stop=True)
                a2 = wp.tile([128, 512], bf16, tag="a2")
                nc.scalar.activation(a2[:, :w], ps2[:, :w], AF.Relu, bias=encNb[:, 1:2])
                ps3 = pM.tile([128, 512], f32, tag="pM")
                nc.tensor.matmul(ps3[:, :w], encNW[:, 1, :], a2[:, :w], start=True, stop=True)
                a3 = wp.tile([128, 512], bf16, tag="a1")
                nc.scalar.activation(a3[:, :w], ps3[:, :w], AF.Relu, bias=encNb[:, 2:3])
                psr = pSp.tile([128, 4, 128], f32, tag="pS")
                for j in range(nt4):
                    nc.tensor.matmul(psr[:, j, :], a3[:, j * 128 : (j + 1) * 128], encNW[:, 2, :], start=True, stop=True)
                s1c = sp.tile([128, 1], f32, tag="s1c")
                nc.scalar.activation(h_own[:, 4 * c : 4 * c + nt4, :], psr[:, :nt4, :], AF.Copy, accum_out=s1c[:])
                scr = wp.tile([128, 512], f32, tag="scr")
                hvv = h_own[:, 4 * c : 4 * c + nt4, :].rearrange("p t f -> p (t f)")
                nc.vector.tensor_tensor(scr[:, :w], hvv, hvv, ALU.mult)
                s2c = sp.tile([128, 1], f32, tag="s2c")
                nc.vector.tensor_reduce(s2c[:], scr[:, :w], AX.X, ALU.add)
                nc.vector.tensor_tensor(hsum[:], hsum[:], s1c[:], ALU.add)
                nc.vector.tensor_tensor(hsq[:], hsq[:], s2c[:], ALU.add)

            # ================= EDGE ENCODER =================
            for c in range(ECH):
                c0 = c * 512
                eat = wp.tile([3, 512], bf16, tag="eat")
                nc.sync.dma_start(eat[:], eaT_d[:, c0 : c0 + 512])
                ps = pM.tile([128, 512], f32, tag="pM")
                nc.tensor.matmul(ps[:], encEW0[:], eat[:], start=True, stop=True)
                a1 = wp.tile([128, 512], bf16, tag="a1")
                nc.scalar.activation(a1[:], ps[:], AF.Relu, bias=encEb[:, 0:1])
                ps2 = pM.tile([128, 512], f32, tag="pM")
                nc.tensor.matmul(ps2[:], encEW[:, 0, :], a1[:], start=True, stop=True)
                a2 = wp.tile([128, 512], bf16, tag="a2")
                nc.scalar.activation(a2[:], ps2[:], AF.Relu, bias=encEb[:, 1:2])
                ps3 = pM.tile([128, 512], f32, tag="pM")
                nc.tensor.matmul(ps3[:], encEW[:, 1, :], a2[:], start=True, stop=True)
                a3 = wp.tile([128, 512], bf16, tag="a1")
                nc.scalar.activation(a3[:], ps3[:], AF.Relu, bias=encEb[:, 2:3])
                ps4 = pM.tile([128, 512], f32, tag="pM")
                nc.tensor.matmul(ps4[:], encEW[:, 2, :], a3[:], start=True, stop=True)
                s1c = sp.tile([128, 1], f32, tag="s1c")
                nc.scalar.activation(e_fm[:, 4 * c : 4 * c + 4, :], ps4[:].rearrange("p (t f) -> p t f", t=4), AF.Copy, accum_out=s1c[:])
                scr = wp.tile([128, 512], f32, tag="scr")
                evv = e_fm[:, 4 * c : 4 * c + 4, :].rearrange("p t f -> p (t f)")
                nc.vector.tensor_tensor(scr[:], evv, evv, ALU.mult)
                s2c = sp.tile([128, 1], f32, tag="s2c")
                nc.vector.tensor_reduce(s2c[:], scr[:], AX.X, ALU.add)
                nc.vector.tensor_tensor(esum[:], esum[:], s1c[:], ALU.add)
                nc.vector.tensor_tensor(esq[:], esq[:], s2c[:], ALU.add)

            # ============ GLOBAL GRAPH-LN STATS ============
            st4 = sp.tile([128, 4], f32, tag="st4")
            for j, t_ in enumerate((hsum, hsq, esum, esq)):
                nc.vector.tensor_copy(st4[:, j : j + 1], t_[:])
            psst = pSp.tile([128, 4, 128], f32, tag="pS")
            nc.tensor.matmul(psst[:4, 0, :1], st4[:], onesK[:], start=True, stop=True)
            stv = sp.tile([4, 1], f32, tag="stv")
            nc.scalar.activation(stv[:], psst[:4, 0, :1], AF.Copy)
            nc.sync.dma_start(sti_d[:], stv[:, 0:1])
            nc.gpsimd.collective_compute(
                "AllReduce", ALU.add, replica_groups=RG, ins=[sti_d[:]], outs=[sto_d[:]]
            )
            st14 = sp.tile([1, 4], f32, tag="st14")
            nc.sync.dma_start(st14[:], sto_d[:])
            psb = pSp.tile([128, 4, 128], f32, tag="pS")
            nc.tensor.matmul(psb[:, 0, :4], ones1[:], st14[:], start=True, stop=True)
            stb = sp.tile([128, 4], f32, tag="stb")
            nc.scalar.activation(stb[:], psb[:, 0, :4], AF.Copy)

            def graph_ln_factors(sumc, sqc, count):
                mu = sp.tile([128, 1], f32, tag="gmu")
                nc.vector.tensor_scalar(mu[:], sumc, 1.0 / count, None, ALU.mult)
                e2 = sp.tile([128, 1], f32, tag="ge2")
                nc.vector.tensor_scalar(e2[:], sqc, 1.0 / count, None, ALU.mult)
                mu2 = sp.tile([128, 1], f32, tag="gmu2")
                nc.scalar.activation(mu2[:], mu[:], AF.Square)
                var = sp.tile([128, 1], f32, tag="gvar")
                nc.vector.tensor_tensor(var[:], e2[:], mu2[:], ALU.subtract)
                sd = sp.tile([128, 1], f32, tag="gsd")
                nc.scalar.activation(sd[:], var[:], AF.Sqrt)
                nc.vector.tensor_scalar(sd[:], sd[:], EPS, None, ALU.add)
                r = sp.tile([128, 1], f32, tag="gr")
                nc.vector.reciprocal(r[:], sd[:])
                nmr = sp.tile([128, 1], f32, tag="gnmr")
                nc.vector.tensor_scalar(nmr[:], mu[:], r[:], -1.0, ALU.mult, ALU.mult)
                return r, nmr

            rh, nmrh = graph_ln_factors(stb[:, 0:1], stb[:, 1:2], float(N) * H)
            re, nmre = graph_ln_factors(stb[:, 2:3], stb[:, 3:4], float(E) * H)

            # apply graph-LN; build hb / hfm / h-table shard
            for c in range(NCH):
                c0 = c * 512
                w = min(512, NPAD - c0)
                nt4 = w // 128
                hv = h_own[:, 4 * c : 4 * c + nt4, :].rearrange("p t f -> p (t f)")
                nc.vector.tensor_scalar(hv, hv, rh[:], nmrh[:], ALU.mult, ALU.add)
                if fl["gln"]:
                    for j in range(nt4):
                        nc.vector.tensor_tensor(h_own[:, 4 * c + j, :], h_own[:, 4 * c + j, :], gNw[:], ALU.mult)
                        nc.vector.tensor_tensor(h_own[:, 4 * c + j, :], h_own[:, 4 * c + j, :], gNb[:], ALU.add)
                nc.scalar.activation(hb[:, 4 * c : 4 * c + nt4, :], h_own[:, 4 * c : 4 * c + nt4, :], AF.Copy)
                nc.sync.dma_start(
                    hsh_d[c0 : c0 + w, :].rearrange("(t p) f -> p t f", p=128),
                    hb[:, 4 * c : 4 * c + nt4, :],
                )
                nc.scalar.dma_start_transpose(
                    hfm[:, 4 * c : 4 * c + nt4, :],
                    hb[:, 4 * c : 4 * c + nt4, :].rearrange("p t f -> p (t f)"),
                )
            for c in range(ECH):
                evc = e_fm[:, 4 * c : 4 * c + 4, :].rearrange("p t f -> p (t f)")
                nc.vector.tensor_scalar(evc, evc, re[:], nmre[:], ALU.mult, ALU.add)
                if fl["gln"]:
                    nc.vector.tensor_scalar(evc, evc, gEw[:], gEb[:], ALU.mult, ALU.add)
            nc.gpsimd.collective_compute(
                "AllGather", ALU.bypass, replica_groups=RG, ins=[hsh_d[:]], outs=[htab_d[:]]
            )
            nc.sync.dma_start(htabl_d[:], htab_d[:])
            if DBG:
                nc.sync.dma_start(dbg_e_d[:], e_fm[:])
                nc.sync.dma_start(dbg_h_d[:], h_own[:])
                dcp = wp.tile([128, NT, 128], f32, tag="dcp")
                nc.scalar.activation(dcp[:], hfm[:], AF.Copy)
                nc.sync.dma_start(dbg_hfm_d[:], dcp[:])

            # ================= MP LAYERS =================
            for l in range(L):
                cur_w = -1
                pagg = None
                pw_sbs = {}
                ntile_w = {}
                for t in range(ET):
                    ntile_w[wsched[t]] = ntile_w.get(wsched[t], 0) + 1
                seen_w = {}

                # -------- edge phase --------
                for c in range(ECH):
                    hsf = gatp.tile([128, 1, 512], bf16, tag="hsf")
                    nc.gpsimd.dma_gather(hsf[:], htabl_d[:], srci[:, c * 32 : c * 32 + 32], 512, 512, 128, transpose=True, queue_num=c % 4)
                    if DBG and l == 0 and c == 0:
                        dcph = wp.tile([128, 4, 128], f32, tag="dcph")
                        nc.scalar.activation(dcph[:], hsf[:, 0, :].rearrange("p (t f) -> p t f", t=4), AF.Copy)
                        nc.sync.dma_start(dbg_hsf_d[:], dcph[:])
                    segB = segp.tile([128, 8, 128], bf16, tag="segB")
                    nc.sync.dma_start(segB[:], segB_d[4 * c : 4 * c + 4].rearrange("t s p f -> p (t s) f"))
                    ebf = wp.tile([128, 512], bf16, tag="ebf")
                    nc.scalar.activation(ebf[:], e_fm[:, 4 * c : 4 * c + 4, :].rearrange("p t f -> p (t f)"), AF.Copy)
                    ps = pM.tile([128, 512], f32, tag="pM")
                    nc.tensor.matmul(ps[:], eW0[:, 3 * l + 1, :], hsf[:, 0, :], start=True, stop=False, skip_group_check=True)
                    nc.tensor.matmul(ps[:], eW0[:, 3 * l + 2, :], ebf[:], start=False, stop=False, skip_group_check=True)
                    for j in range(4):
                        t = 4 * c + j
                        wd = wsched[t]
                        if wd not in pw_sbs:
                            psw = pWp.tile([128, 128], f32, tag="pW")
                            nc.tensor.matmul(psw[:], hfm[:, wd, :], eW0[:, 3 * l, :], start=True, stop=True)
                            pw_sbs[wd] = wp.tile([128, 128], bf16, tag="pwsb", name="pwsb")
                            nc.scalar.activation(pw_sbs[wd][:], psw[:], AF.Copy)
                        nc.tensor.matmul(ps[:, j * 128 : (j + 1) * 128], pw_sbs[wd][:], segB[:, 2 * j + 1, :], start=False, stop=True, skip_group_check=True)
                    a1 = wp.tile([128, 512], bf16, tag="a1")
                    nc.scalar.activation(a1[:], ps[:], AF.Relu, bias=eb[:, 3 * l : 3 * l + 1])
                    ps2 = pM.tile([128, 512], f32, tag="pM")
                    nc.tensor.matmul(ps2[:], eWs0[:, l, :], a1[:], start=True, stop=True)
                    a2 = wp.tile([128, 512], bf16, tag="a2")
                    nc.scalar.activation(a2[:], ps2[:], AF.Relu, bias=eb[:, 3 * l + 1 : 3 * l + 2])
                    ps3 = pSp.tile([128, 4, 128], f32, tag="pS")
                    for j in range(4):
                        nc.tensor.matmul(ps3[:, j, :], a2[:, j * 128 : (j + 1) * 128], eWs1[:, l, :], start=True, stop=True)
                    # batched row-LN stats over the 4 tiles
                    p3f = ps3[:].rearrange("p t f -> p (t f)")
                    s1 = sp.tile([128, 4], f32, tag="s1")
                    nc.vector.tensor_reduce(s1[:], ps3[:], AX.X, ALU.add)
                    scr = wp.tile([128, 512], f32, tag="scr")
                    nc.scalar.activation(scr[:], p3f, AF.Square)
                    s2 = sp.tile([128, 4], f32, tag="s2")
                    nc.vector.tensor_reduce(s2[:], scr[:].rearrange("p (t f) -> p t f", t=4), AX.X, ALU.add)
                    mu = sp.tile([128, 4], f32, tag="mu")
                    nc.vector.tensor_scalar(mu[:], s1[:], 1.0 / 128, None, ALU.mult)
                    es2 = sp.tile([128, 4], f32, tag="es2")
                    nc.vector.tensor_scalar(es2[:], s2[:], 1.0 / 128, None, ALU.mult)
                    mu2 = sp.tile([128, 4], f32, tag="mu2")
                    nc.vector.tensor_tensor(mu2[:], mu[:], mu[:], ALU.mult)
                    var = sp.tile([128, 4], f32, tag="var")
                    nc.vector.tensor_tensor(var[:], es2[:], mu2[:], ALU.subtract)
                    sd = sp.tile([128, 4], f32, tag="sd")
                    nc.scalar.activation(sd[:], var[:], AF.Sqrt, bias=epsA[:])
                    rs = sp.tile([128, 4], f32, tag="rs")
                    nc.vector.reciprocal(rs[:], sd[:])
                    nmr = sp.tile([128, 4], f32, tag="nmr")
                    nc.vector.tensor_tensor(nmr[:], mu[:], rs[:], ALU.mult)
                    nc.vector.tensor_scalar(nmr[:], nmr[:], -1.0, None, ALU.mult)
                    tmpb4 = wp.tile([128, 4, 128], bf16, tag="tmpb4")
                    for j in range(4):
                        t = 4 * c + j
                        wd = wsched[t]
                        nc.vector.tensor_scalar(tmpb4[:, j, :], ps3[:, j, :], rs[:, j : j + 1], nmr[:, j : j + 1], ALU.mult, ALU.add)
                        if fl["eln"]:
                            tmpf = wp.tile([128, 128], f32, tag="tmpf")
                            nc.vector.tensor_scalar(tmpf[:], ps3[:, j, :], rs[:, j : j + 1], nmr[:, j : j + 1], ALU.mult, ALU.add)
                            nc.vector.tensor_tensor(tmpf[:], tmpf[:], elnw[:, l, :], ALU.mult)
                            nc.vector.tensor_tensor(tmpf[:], tmpf[:], elnb[:, l, :], ALU.add)
                            nc.vector.tensor_copy(tmpb4[:, j, :], tmpf[:])
                        if wd != cur_w:
                            if cur_w >= 0:
                                nc.vector.tensor_tensor(agf[:, cur_w, :], pagg[:], icntb[:, cur_w, :], ALU.mult)
                            cur_w = wd
                            pagg = pAp.tile([128, 128], f32, tag="pA")
                            seen_w[wd] = 0
                        nc.tensor.matmul(pagg[:], tmpb4[:, j, :], segB[:, 2 * j, :], start=(seen_w[wd] == 0), stop=(seen_w[wd] == ntile_w[wd] - 1), skip_group_check=True)
                        seen_w[wd] += 1
                    ptr = pTp.tile([128, 4, 128], bf16, tag="pT")
                    for j in range(4):
                        nc.tensor.transpose(ptr[:, j, :], tmpb4[:, j, :], ident[:])
                    ev4 = e_fm[:, 4 * c : 4 * c + 4, :].rearrange("p t f -> p (t f)")
                    nc.vector.tensor_tensor(ev4, ev4, ptr[:].rearrange("p t f -> p (t f)"), ALU.add)
                # close last window
                nc.vector.tensor_tensor(agf[:, cur_w, :], pagg[:], icntb[:, cur_w, :], ALU.mult)
                if DBG and l == 0:
                    dcp2 = wp.tile([128, NT, 128], f32, tag="dcp")
                    nc.scalar.activation(dcp2[:], agf[:], AF.Copy)
                    nc.sync.dma_start(dbg_agf_d[:], dcp2[:])
                    nc.sync.dma_start(dbg_e1_d[:], e_fm[:])

                # -------- node phase --------
                for c in range(NCH):
                    c0 = c * 512
                    w = min(512, NPAD - c0)
                    nt4 = w // 128
                    ps = pM.tile([128, 512], f32, tag="pM")
                    nc.tensor.matmul(ps[:, :w], nW0[:, 2 * l, :], hfm[:, 4 * c : 4 * c + nt4, :].rearrange("p t f -> p (t f)"), start=True, stop=False, skip_group_check=True)
                    nc.tensor.matmul(ps[:, :w], nW0[:, 2 * l + 1, :], agf[:, 4 * c : 4 * c + nt4, :].rearrange("p t f -> p (t f)"), start=False, stop=True, skip_group_check=True)
                    a1 = wp.tile([128, 512], bf16, tag="a1")
                    nc.scalar.activation(a1[:, :w], ps[:, :w], AF.Relu, bias=nb[:, 3 * l : 3 * l + 1])
                    ps2 = pM.tile([128, 512], f32, tag="pM")
                    nc.tensor.matmul(ps2[:, :w], nWs0[:, l, :], a1[:, :w], start=True, stop=True)
                    a2 = wp.tile([128, 512], bf16, tag="a2")
                    nc.scalar.activation(a2[:, :w], ps2[:, :w], AF.Relu, bias=nb[:, 3 * l + 1 : 3 * l + 2])
                    ps3 = pSp.tile([128, 4, 128], f32, tag="pS")
                    for j in range(nt4):
                        nc.tensor.matmul(ps3[:, j, :], a2[:, j * 128 : (j + 1) * 128], nWs1[:, l, :], start=True, stop=True)
                    s1 = sp.tile([128, 4], f32, tag="s1")
                    nc.vector.tensor_reduce(s1[:, :nt4], ps3[:, :nt4, :], AX.X, ALU.add)
                    scr = wp.tile([128, 512], f32, tag="scr")
                    p3f = ps3[:, :nt4, :].rearrange("p t f -> p (t f)")
                    nc.scalar.activation(scr[:, :w], p3f, AF.Square)
                    s2 = sp.tile([128, 4], f32, tag="s2")
                    nc.vector.tensor_reduce(s2[:, :nt4], scr[:, :w].rearrange("p (t f) -> p t f", t=nt4), AX.X, ALU.add)
                    mu = sp.tile([128, 4], f32, tag="mu")
                    nc.vector.tensor_scalar(mu[:, :nt4], s1[:, :nt4], 1.0 / 128, None, ALU.mult)
                    es2 = sp.tile([128, 4], f32, tag="es2")
                    nc.vector.tensor_scalar(es2[:, :nt4], s2[:, :nt4], 1.0 / 128, None, ALU.mult)
                    mu2 = sp.tile([128, 4], f32, tag="mu2")
                    nc.vector.tensor_tensor(mu2[:, :nt4], mu[:, :nt4], mu[:, :nt4], ALU.mult)
                    var = sp.tile([128, 4], f32, tag="var")
                    nc.vector.tensor_tensor(var[:, :nt4], es2[:, :nt4], mu2[:, :nt4], ALU.subtract)
                    sd = sp.tile([128, 4], f32, tag="sd")
                    nc.scalar.activation(sd[:, :nt4], var[:, :nt4], AF.Sqrt, bias=epsA[:])
                    rs = sp.tile([128, 4], f32, tag="rs")
                    nc.vector.reciprocal(rs[:, :nt4], sd[:, :nt4])
                    nmr = sp.tile([128, 4], f32, tag="nmr")
                    nc.vector.tensor_tensor(nmr[:, :nt4], mu[:, :nt4], rs[:, :nt4], ALU.mult)
                    nc.vector.tensor_scalar(nmr[:, :nt4], nmr[:, :nt4], -1.0, None, ALU.mult)
                    upd = wp.tile([128, 4, 128], f32, tag="upd")
                    for j in range(nt4):
                        nc.vector.tensor_scalar(upd[:, j, :], ps3[:, j, :], rs[:, j : j + 1], nmr[:, j : j + 1], ALU.mult, ALU.add)
                        if fl["nln"]:
                            nc.vector.tensor_tensor(upd[:, j, :], upd[:, j, :], nlnw[:, l, :], ALU.mult)
                            nc.vector.tensor_tensor(upd[:, j, :], upd[:, j, :], nlnb[:, l, :], ALU.add)
                    hv = h_own[:, 4 * c : 4 * c + nt4, :].rearrange("p t f -> p (t f)")
                    nc.gpsimd.tensor_tensor(hv, hv, upd[:, :nt4, :].rearrange("p t f -> p (t f)"), ALU.add)
                    nc.scalar.activation(hb[:, 4 * c : 4 * c + nt4, :], h_own[:, 4 * c : 4 * c + nt4, :], AF.Copy)
                    if l < L - 1:
                        nc.sync.dma_start(
                            hsh_d[c0 : c0 + w, :].rearrange("(t p) f -> p t f", p=128),
                            hb[:, 4 * c : 4 * c + nt4, :],
                        )
                    nc.scalar.dma_start_transpose(
                        hfm[:, 4 * c : 4 * c + nt4, :],
                        hb[:, 4 * c : 4 * c + nt4, :].rearrange("p t f -> p (t f)"),
                    )
                if DBG and l == 0:
                    nc.sync.dma_start(dbg_h1_d[:], h_own[:])
                if l < L - 1:
                    nc.gpsimd.collective_compute(
                        "AllGather", ALU.bypass, replica_groups=RG, ins=[hsh_d[:]], outs=[htab_d[:]]
                    )
                    nc.sync.dma_start(htabl_d[:], htab_d[:])

            # ================= DECODER =================
            for c in range(NCH):
                c0 = c * 512
                w = min(512, NPAD - c0)
                nt4 = w // 128
                hfc = hfm[:, 4 * c : 4 * c + nt4, :].rearrange("p t f -> p (t f)")
                ps = pM.tile([128, 512], f32, tag="pM")
                nc.tensor.matmul(ps[:, :w], decW[:, 0, :], hfc, start=True, stop=True)
                a1 = wp.tile([128, 512], bf16, tag="a1")
                nc.scalar.activation(a1[:, :w], ps[:, :w], AF.Relu, bias=decb[:, 0:1])
                ps2 = pM.tile([128, 512], f32, tag="pM")
                nc.tensor.matmul(ps2[:, :w], decW[:, 1, :], a1[:, :w], start=True, stop=True)
                a2 = wp.tile([128, 512], bf16, tag="a2")
                nc.scalar.activation(a2[:, :w], ps2[:, :w], AF.Relu, bias=decb[:, 1:2])
                ps3 = pM.tile([128, 512], f32, tag="pM")
                nc.tensor.matmul(ps3[:, :w], decW[:, 2, :], a2[:, :w], start=True, stop=True)
                a3 = wp.tile([128, 512], bf16, tag="a1")
                nc.scalar.activation(a3[:, :w], ps3[:, :w], AF.Relu, bias=decb[:, 2:3])
                psd = pM.tile([128, 512], f32, tag="pM")
                nc.tensor.matmul(psd[:3, :w], decWl[:], a3[:, :w], start=True, stop=True)
                ot = wp.tile([3, 512], f32, tag="ot")
                if fl["decbl"]:
                    nc.vector.tensor_scalar(ot[:, :w], psd[:3, :w], 1.0, decbl[:], ALU.mult, ALU.add)
                else:
                    nc.scalar.activation(ot[:, :w], psd[:3, :w], AF.Copy)
                nc.sync.dma_start(out_d[:, c0 : c0 + w], ot[:, :w])

    nc.compile()
    return nc


def _prep(inputs, cfg):
    """Host-side sharding/index prep. Returns in_maps list."""
    N, E, L = cfg["N"], cfg["E"], cfg["L"]
    NPC, NPAD, ECP = cfg["NPC"], cfg["NPAD"], cfg["EC_PAD"]
    wsched = cfg["wsched"]
    ET = ECP // 128
    NW = NPAD // 128
    f = lambda k: np.asarray(inputs[k], np.float32)
    b = lambda a: np.ascontiguousarray(a).astype(ml_dtypes.bfloat16)

    ei = np.asarray(inputs["edge_index"])
    src_g, dst_g = ei[0].astype(np.int64), ei[1].astype(np.int64)
    ea = f("edge_attr")
    x = f("x")
    cnt = np.bincount(dst_g, minlength=N).astype(np.float32)
    icnt_full = 1.0 / np.maximum(cnt, 1.0)

    tblrow = lambda g: (g // NPC) * NPAD + (g % NPC)

    pos = {}
    for t, wd in enumerate(wsched):
        pos.setdefault(wd, []).append(t)

    order = np.argsort(dst_g, kind="stable")
    in_maps = []
    shared = None
    for c in range(NC):
        lo, hi = c * NPC, (c + 1) * NPC
        sel = order[(dst_g[order] >= lo) & (dst_g[order] < hi)]
        dl = dst_g[sel] - lo           # local dst, ascending
        win = dl // 128
        srcv = np.zeros(ECP, np.int64)
        eav = np.zeros((ECP, 3), np.float32)
        seg = np.zeros((ET, 128, 128), np.float32)
        for wd in range(NW):
            idxs = np.where(win == wd)[0]
            tiles = pos.get(wd, [])
            assert len(idxs) <= len(tiles) * 128, (c, wd, len(idxs), len(tiles))
            for k, i in enumerate(idxs):
                t = tiles[k // 128]
                r = k % 128
                g = t * 128 + r
                e_ = sel[i]
                srcv[g] = src_g[e_]
                eav[g] = ea[e_]
                seg[t, r, dl[i] - 128 * wd] = 1.0
        segB = np.stack([seg, seg.transpose(0, 2, 1)], axis=1)  # [ET, 2, 128, 128]
        icnt_c = np.ones(NPAD, np.float32)
        icnt_c[:NPC] = icnt_full[lo:hi]
        icntb = np.tile(icnt_c.reshape(1, NW, 128), (128, 1, 1)).copy()
        xT = np.zeros((5, NPAD), np.float32)
        xT[:, :NPC] = x[lo:hi].T
        eaT = eav.T.copy()
        m = {
            "xT": b(xT), "eaT": b(eaT),
            "srci": _wrap_idx(tblrow(srcv).astype(np.int16)),
            "segB": b(segB), "icntb": icntb,
        }
        if shared is None:
            shared = {
                "ident": b(np.eye(128)),
                "ones1": np.ones((1, 128), np.float32),
                "onesK": np.ones((128, 1), np.float32),
                "encNW0": b(f("encN_W0")), "encNW": b(f("encN_Ws")),
                "encEW0": b(f("encE_W0")), "encEW": b(f("encE_Ws")),
                "eW0": b(f("eW0").reshape(L, 3, 128, 128)),
                "eWs0": b(f("eWs")[:, 0]), "eWs1": b(f("eWs")[:, 1]),
                "nW0": b(f("nW0").reshape(L, 2, 128, 128)),
                "nWs0": b(f("nWs")[:, 0]), "nWs1": b(f("nWs")[:, 1]),
                "decW": b(np.stack([f("dec_W0"), f("dec_Ws")[0], f("dec_Ws")[1]])),
                "decWl": b(f("dec_Wl")),
                "encNb": f("encN_bs").T.copy(), "encEb": f("encE_bs").T.copy(),
                "eb": f("ebs").reshape(L * 3, 128).T.copy(),
                "nb": f("nbs").reshape(L * 3, 128).T.copy(),
                "decb": f("dec_bs").T.copy(),
            }
            flg = cfg["flags"]
            if flg["eln"]:
                shared["elnw"] = np.tile(f("elnw")[:, None, :], (1, 128, 1))
                shared["elnb"] = np.tile(f("elnb")[:, None, :], (1, 128, 1))
            if flg["nln"]:
                shared["nlnw"] = np.tile(f("nlnw")[:, None, :], (1, 128, 1))
                shared["nlnb"] = np.tile(f("nlnb")[:, None, :], (1, 128, 1))
            if flg["gln"]:
                shared["gNw"] = np.tile(f("encN_lnw")[None, :], (128, 1))
                shared["gNb"] = np.tile(f("encN_lnb")[None, :], (128, 1))
                shared["gEw"] = f("encE_lnw").reshape(128, 1).copy()
                shared["gEb"] = f("encE_lnb").reshape(128, 1).copy()
            if flg["decbl"]:
                shared["decbl"] = f("dec_bl").reshape(3, 1).copy()
        m.update(shared)
        in_maps.append(m)
    return in_maps


def make_cfg(inputs):
    N = np.asarray(inputs["x"]).shape[0]
    E = np.asarray(inputs["edge_index"]).shape[1]
    L = np.asarray(inputs["eW0"]).shape[0]
    NPC = N // NC
    NPAD = ((NPC + 127) // 128) * 128
    NW = NPAD // 128
    ei = np.asarray(inputs["edge_index"])
    dst = ei[1].astype(np.int64)
    tw = []
    for wd in range(NW):
        mx = 1
        for c in range(NC):
            lo = c * NPC
            nwin = int(((dst >= lo + wd * 128) & (dst < min(lo + (wd + 1) * 128, lo + NPC))).sum())
            mx = max(mx, (nwin + 127) // 128)
        tw.append(mx)
    wsched = []
    for wd in range(NW):
        wsched += [wd] * tw[wd]
    while (len(wsched) * 128) % 512:
        wsched.append(NW - 1)
    flags = {
        "eln": bool(np.any(np.asarray(inputs["elnw"]) != 1) or np.any(np.asarray(inputs["elnb"]) != 0)),
        "nln": bool(np.any(np.asarray(inputs["nlnw"]) != 1) or np.any(np.asarray(inputs["nlnb"]) != 0)),
        "gln": bool(
            np.any(np.asarray(inputs["encN_lnw"]) != 1) or np.any(np.asarray(inputs["encN_lnb"]) != 0)
            or np.any(np.asarray(inputs["encE_lnw"]) != 1) or np.any(np.asarray(inputs["encE_lnb"]) != 0)
        ),
        "decbl": bool(np.any(np.asarray(inputs["dec_bl"]) != 0)),
    }
    return {
        "N": N, "E": E, "L": L, "NPC": NPC, "NPAD": NPAD,
        "EC_PAD": len(wsched) * 128, "wsched": wsched, "flags": flags,
    }


_CACHE = {}


def kernel(**inputs) -> np.ndarray:
    cfg = make_cfg(inputs)
    key = (cfg["N"], cfg["E"], cfg["L"], cfg["EC_PAD"], tuple(sorted(cfg["flags"].items())), os.environ.get("KDBG", "0"))
    if key not in _CACHE:
        _CACHE[key] = build(cfg)
    nc = _CACHE[key]
    in_maps = _prep(inputs, cfg)
    res = run_bass_kernel_spmd(nc, in_maps, list(range(NC))).results
    NPC = cfg["NPC"]
    out = np.concatenate([res[c]["out"][:, :NPC].T for c in range(NC)], axis=0)
    return np.ascontiguousarray(out).astype(np.float32)
